# revision 37
# baseline (speedup 1.0000x reference)
"""Trainium2 Bass kernel: AdaptiveAttentionFusion, pure data-parallel on 8 NeuronCores.

Strategy:
  - Shard batch B=16384 across 8 cores (2048 rows each); weights replicated.
  - Host-side folding: input projections folded into QKV / MLP weights so raw
    inputs feed all matmuls. k/v biases eliminated algebraically (constant-
    over-s score shifts cancel in softmax; mean v-bias folds into bo).
  - qkv + MLP-hidden matmuls run fp8e4 DoubleRow (2 k-chunks per instruction);
    weights pre-scaled x32 into e4m3's normal range. The x32 PSUMs are free to
    handle: LN is scale-invariant, softmax exps fold 1/32 into their scale,
    and v descales at eviction. Output-critical projections (map/frontier/
    cross residuals, Wo, gate) stay fp16.
  - rawT / ctxT via DMA XBAR transpose (2-byte); rawT8 cast on ACT.
  - Scores: 9 fp16 2x-mode muls into per-q packed tiles, k-dim log-folded
    128->32 with two TT adds, then quarter-size 1x reduces.
  - All elementwise work on DVE/ACT; GpSimd only takes the z-mul. (Measured:
    GpSimd TT ~1.1us per [128,512] op both poisons critical chains and, when
    loaded, stalls the pipeline - keep it nearly empty.)
  - Emission pipeline: A1(i) / A2(i-1) / rawT-DMA(i+2) / B1(i-3) / B2(i-4);
    ctxT transposes enqueue on Sync ahead of next rawT loads so the o-proj
    stationaries never queue behind slot-blocked input DMAs.
  - LN1 stats batched: shared (mean,var)x3 tile, one strided Ln + one Exp +
    one nmr stt for all three modalities.
"""
import os
import numpy as np

import concourse.bacc as bacc
import concourse.bass as bass
import concourse.tile as tile
from concourse import mybir
from concourse.bass_utils import run_bass_kernel_spmd
from concourse.masks import make_identity

D, H, KD = 512, 4, 128
NCORES = 8
EPS = 1e-6
P = 128
F16 = mybir.dt.float16
F32 = mybir.dt.float32
F8 = mybir.dt.float8e4
DR = mybir.MatmulPerfMode.DoubleRow
AF = mybir.ActivationFunctionType
OP = mybir.AluOpType
AX = mybir.AxisListType
SCL = 32.0       # fp8 weight pre-scale (power of 2, exact)
ISCL = 1.0 / SCL

# op->engine assignment knobs (tuned against the trace)
SCORES_ON_GPSIMD = False
REDUCES_ON_GPSIMD = False
Z_ON_GPSIMD = True
CTX_MULS_ON_GPSIMD = False
WEIGHTED_ON_GPSIMD = False  # tensor_scalar-ptr ops are not supported on Pool/GpSimd
USE_TTR = False

LAST_EXEC_TIME_NS = None
LAST_RESULTS = None


def ts(i, n=P):
    return slice(i * n, (i + 1) * n)


def _fold(inp):
    """Fold input projections into downstream weights."""
    Ws = [inp['Wf'], inp['Wr'], inp['Wm']]
    bs = [inp['bf'], inp['br'], inp['bm']]
    Wqkv = np.concatenate([inp['Wq'].reshape(D, D), inp['Wk'].reshape(D, D),
                           inp['Wv'].reshape(D, D)], axis=1)          # [512, 1536]
    bqkv = np.concatenate([inp['bq'].reshape(-1), inp['bk'].reshape(-1),
                           inp['bv'].reshape(-1)])                    # [1536]
    f = {}
    f8 = mybir.dt.np(mybir.dt.float8e4)
    # All rawT-sourced matmuls run fp8e4 DoubleRow. Weights are pre-scaled by
    # SCL (=32, exact in fp16/fp8) so their ~0.01-magnitude entries land in
    # e4m3's normal range; the resulting PSUMs carry a uniform x32 which is
    # (a) invariant under LN, (b) divided out in the softmax-exp scales, and
    # (c) divided out of v at eviction and of x_m at the z-combine.
    f['wproj'] = np.concatenate(Ws, axis=1).astype(np.float16)        # [512, 1536]
    f['bm'] = bs[2][None, :].astype(np.float16)                       # [1, 512]
    # v feature order interleaved (h*128+k)->(k*4+h) so the per-head attn
    # broadcast in the ctx muls is inner-contiguous; wo rows match.
    perm = (np.arange(D).reshape(H, KD).T).reshape(-1)                # perm[k*4+h] = h*128+k
    Wqkv = Wqkv.copy()
    Wqkv[:, 2 * D:3 * D] = Wqkv[:, 2 * D + perm]
    f['wqkv'] = (np.concatenate([Ws[s] @ Wqkv for s in range(3)], axis=1) * SCL).astype(f8)  # [512, 4608]
    bqkv_full = np.stack([bs[s] @ Wqkv + bqkv for s in range(3)])     # [3, 1536]
    # k-bias: constant-over-s score shifts cancel in the softmax over s
    # (the tiny per-s folded-input-bias delta is ~1e-3 of the logit std).
    # v-bias: softmax weights sum to 1 over s, so mean_s(bv_s) flows through
    # ctx exactly; fold it into bo below. Only the q-bias matmul remains.
    bv_bar = bqkv_full[:, 2 * D:3 * D].mean(axis=0)                   # [512]
    bo_comp = bv_bar @ inp['Wo'].reshape(D, D)                        # [512]
    f['bqkv'] = (bqkv_full * SCL).reshape(-1)[None, :].astype(np.float16)
    A = [Ws[s] @ inp['Wa1'][s * D:(s + 1) * D] for s in range(3)]
    f['wh'] = (np.concatenate(A, axis=1) * SCL).astype(f8)            # [512, 768]
    bh = (inp['ba1'] + sum(bs[s] @ inp['Wa1'][s * D:(s + 1) * D] for s in range(3)))
    f['bht'] = (bh * SCL).reshape(2, P).T.astype(np.float32).copy()   # [128, 2]
    f['wo'] = inp['Wo'].reshape(D, D)[perm].astype(np.float16)
    f['bxo'] = np.concatenate([inp['bf'] + inp['bo'] + bo_comp,
                               inp['br'] + inp['bo'] + bo_comp,
                               inp['bm'] + inp['bo'] + bo_comp])[None, :].astype(np.float16)
    f['wg'] = inp['Wg'].astype(np.float16)
    f['bg'] = inp['bg'][None, :].astype(np.float16)
    f['wa2t'] = inp['Wa2'].astype(np.float16)  # [256, 3]
    f['ba2'] = (inp['ba2'] * SCL)[None, :].astype(np.float16)
    aff = np.stack([inp['gamma1'], inp['beta1'], inp['gamma2'], inp['beta2']]).astype(np.float32)
    return f, aff


def _pin_act_table(nc):
    """Instance-level override of insert_act_table_loads: make
    natural_log_exp_and_others the only selectable ACT table set, so the
    kernel never thrashes table loads (it covers exp/ln/relu/copy/identity)."""
    import types
    import bass_rust as _bass_rust
    from concourse.hw_specs import get_activation_tables

    def patched(self):
        has_activation = any(
            isinstance(i, mybir.InstActivation)
            for b in self.main_func.blocks
            for i in b.instructions
        )
        if not has_activation:
            return
        tables = [
            (name, fns if name == "natural_log_exp_and_others" else set())
            for name, fns in get_activation_tables(self.m.arch).items()
        ]
        _bass_rust.insert_act_table_loads(self, tables)

    nc.insert_act_table_loads = types.MethodType(patched, nc)


def _build(R, need_aff1, need_aff2):
    ntiles = R // P
    nc = bacc.Bacc()
    _pin_act_table(nc)

    x_ext = [nc.declare_dram_parameter(n, [R, D], F16, isOutput=False)
             for n in ("frontier", "cross_robot", "map_feat")]
    wproj_d = nc.declare_dram_parameter("wproj", [D, 3 * D], F16, isOutput=False)
    bm_d = nc.declare_dram_parameter("bm", [1, D], F16, isOutput=False)
    wqkv_d = nc.declare_dram_parameter("wqkv", [D, 9 * D], F8, isOutput=False)
    bqkv_d = nc.declare_dram_parameter("bqkv", [1, 9 * D], F16, isOutput=False)
    wh_d = nc.declare_dram_parameter("wh", [D, 768], F8, isOutput=False)
    bht_d = nc.declare_dram_parameter("bht", [P, 2], F32, isOutput=False)
    wo_d = nc.declare_dram_parameter("wo", [D, D], F16, isOutput=False)
    wg_d = nc.declare_dram_parameter("wg", [D, D], F16, isOutput=False)
    bg_d = nc.declare_dram_parameter("bg", [1, D], F16, isOutput=False)
    bxo_d = nc.declare_dram_parameter("bxo", [1, 3 * D], F16, isOutput=False)
    wa2t_d = nc.declare_dram_parameter("wa2t", [256, 3], F16, isOutput=False)
    ba2_d = nc.declare_dram_parameter("ba2", [1, 3], F16, isOutput=False)
    aff_d = None
    if need_aff1 or need_aff2:
        aff_d = nc.declare_dram_parameter("aff", [4, D], F32, isOutput=False)
    out_ext = nc.declare_dram_parameter("out", [R, D], F32, isOutput=True)

    def bcast(ap, parts=P):
        """Partition-broadcast DMA source AP (stride-0 partition dim)."""
        return bass.AP(tensor=ap.tensor, offset=ap.offset, ap=[[0, parts]] + list(ap.ap))

    with tile.TileContext(nc) as tc:
        import contextlib
        with contextlib.ExitStack() as ctx:
            const = ctx.enter_context(tc.tile_pool(name="const", bufs=1))
            p_rt = ctx.enter_context(tc.tile_pool(name="p_rt", bufs=6))
            p_x = ctx.enter_context(tc.tile_pool(name="p_x", bufs=6))
            p_qkv = ctx.enter_context(tc.tile_pool(name="p_qkv", bufs=3))
            p_ctx = ctx.enter_context(tc.tile_pool(name="p_ctx", bufs=2))
            p_y = ctx.enter_context(tc.tile_pool(name="p_y", bufs=3))
            p_w = ctx.enter_context(tc.tile_pool(name="p_w", bufs=3))
            p_sm = ctx.enter_context(tc.tile_pool(name="p_sm", bufs=2))
            ps_t16 = ctx.enter_context(tc.tile_pool(name="ps_t16", bufs=1, space="PSUM"))
            ps_mm = ctx.enter_context(tc.tile_pool(name="ps_mm", bufs=7, space="PSUM"))

            # ---- constants (ordered by first use) ----
            ident16 = const.tile([P, P], F16)
            make_identity(nc, ident16)
            # full-width stationary of 1/128 (exact in fp16) keeps every
            # bias matmul FWL-eligible, so the next matmul's LDWEIGHTS
            # prefetches cleanly (K=1 ones-row stationaries cost +107ns on
            # the following matmul)
            onesd = const.tile([P, P], F16)
            nc.vector.memset(onesd, 1.0 / P)
            eps_sb = const.tile([P, 1], F32)
            nc.vector.memset(eps_sb, EPS)

            # XBAR 3D transpose writes rt[p, c, r] = x[r, c*128+p]: chunk-major
            # feature order, matching the plain "(c p)" weight row layout.
            wproj_sb = const.tile([P, 4, 3 * D], F16)
            nc.sync.dma_start(out=wproj_sb, in_=wproj_d[:, :].rearrange("(c p) f -> p c f", p=P))
            wqkv_sb = const.tile([P, 4, 9 * D], F8)
            for c in range(4):
                nc.sync.dma_start(out=wqkv_sb[:, c, :], in_=wqkv_d[c * P:(c + 1) * P, :])
            wh_sb = const.tile([P, 4, 768], F8)
            nc.sync.dma_start(out=wh_sb, in_=wh_d[:, :].rearrange("(c p) f -> p c f", p=P))
            wa2_sb = const.tile([P, 2, 3], F16)
            nc.sync.dma_start(out=wa2_sb, in_=wa2t_d[:, :].rearrange("(c p) j -> p c j", p=P))
            wo_sb = const.tile([P, 4, D], F16)
            nc.sync.dma_start(out=wo_sb, in_=wo_d[:, :].rearrange("(c p) f -> p c f", p=P))
            wg_sb = const.tile([P, 4, D], F16)
            nc.sync.dma_start(out=wg_sb, in_=wg_d[:, :].rearrange("(c p) f -> p c f", p=P))
            bm_sb = const.tile([P, D], F16)
            nc.sync.dma_start(out=bm_sb, in_=bcast(bm_d[:, :]))
            bqkv_sb = const.tile([P, 9 * D], F16)
            nc.sync.dma_start(out=bqkv_sb, in_=bcast(bqkv_d[:, :]))
            bht_sb = const.tile([P, 2], F32)
            nc.sync.dma_start(out=bht_sb, in_=bht_d[:, :])
            bg_sb = const.tile([P, D], F16)
            nc.sync.dma_start(out=bg_sb, in_=bcast(bg_d[:, :]))
            bxo_sb = const.tile([P, 3 * D], F16)
            nc.sync.dma_start(out=bxo_sb, in_=bcast(bxo_d[:, :]))
            ba2_sb = const.tile([P, 3], F16)
            nc.sync.dma_start(out=ba2_sb, in_=bcast(ba2_d[:, :]))
            aff_sb = None
            if aff_d is not None:
                aff_sb = const.tile([P, 4, D], F32)
                nc.sync.dma_start(out=aff_sb, in_=bcast(aff_d[:, :]))

            isk = float(1.0 / np.sqrt(KD))

            def ln_stats(zin_ap):
                """LN stats: bn_stats/aggr on DVE, rsqrt via ln/exp on ScalarE."""
                st6 = p_sm.tile([P, 6], F32, name="st6", tag="st6", bufs=6)
                nc.vector.bn_stats(out=st6[:], in_=zin_ap)
                mv = p_sm.tile([P, 2], F32, name="mv", tag="mv", bufs=6)
                nc.vector.bn_aggr(out=mv[:], in_=st6[:])
                lnt = p_sm.tile([P, 1], F32, name="lnt", tag="lnt", bufs=6)
                nc.scalar.activation(out=lnt[:], in_=mv[:, 1:2], func=AF.Ln,
                                     bias=eps_sb[:], scale=1.0)
                rstd = p_sm.tile([P, 1], F32, name="rstd", tag="rstd", bufs=6)
                nc.scalar.activation(out=rstd[:], in_=lnt[:], func=AF.Exp, scale=-0.5)
                nmr = p_sm.tile([P, 1], F32, name="nmr", tag="nmr", bufs=6)
                nc.vector.scalar_tensor_tensor(out=nmr[:], in0=mv[:, 0:1], scalar=-1.0,
                                               in1=rstd[:], op0=OP.mult, op1=OP.mult)
                return rstd, nmr

            def ln_apply(zin_ap, zout, rstd, nmr, aff_idx):
                """zout = zin*rstd + nmr (ScalarE), then optional gamma/beta."""
                if isinstance(rstd, tile.Tile):
                    rstd, nmr = rstd[:], nmr[:]
                nc.scalar.activation(out=zout[:], in_=zin_ap, func=AF.Identity,
                                     scale=rstd, bias=nmr)
                if aff_idx is not None:
                    nc.vector.tensor_mul(out=zout[:], in0=zout[:], in1=aff_sb[:, aff_idx, :])
                    nc.vector.tensor_add(out=zout[:], in0=zout[:], in1=aff_sb[:, aff_idx + 1, :])

            def emitDMA(i):
                """Hardware XBAR-transposed loads: rawT[s][p, c, r] = x[r0+r, p*4+c]."""
                r0 = i * P
                rawT = []
                for s in range(3):
                    rt = p_rt.tile([P, 4, P], F16, name=f"rawT{s}", tag=f"rawT{s}", bufs=6)
                    nc.sync.dma_start(out=rt[:, :, :], in_=x_ext[s][r0:r0 + P, :],
                                      transpose=True)
                    rawT.append(rt)
                return rawT

            def emitA1(i, rawT):
                """Matmul flood: map proj, qkv, MLP, adaptive logits, score
                products + reduces."""
                r0 = i * P

                # fp8 casts of the transposed raws (map first: proj-m uses it
                # immediately); split ACT/DVE so they run concurrently.
                rawT8 = [None, None, None]
                for s_ in (0, 1, 2):
                    rt8 = p_rt.tile([P, 4, P], F8, name=f"rawT8_{s_}",
                                    tag=f"rawT8_{s_}", bufs=5)
                    nc.scalar.copy(out=rt8[:], in_=rawT[s_][:])
                    rawT8[s_] = rt8

                # map-feat projection only (frontier/cross fold into o-psum in B)
                # (fp16: mp flows straight into the output z; fp8 is too lossy)
                ps = ps_mm.tile([P, D], F32, name="mm", tag="mm")
                for c in range(4):
                    nc.tensor.matmul(ps[:], lhsT=rawT[2][:, c, :],
                                     rhs=wproj_sb[:, c, ts(2, D)],
                                     start=(c == 0), stop=False)
                nc.tensor.matmul(ps[:], lhsT=onesd[:, :], rhs=bm_sb[:, :],
                                 start=False, stop=True)
                x_m = p_x.tile([P, D], F16, name="xm", tag="xm")
                nc.scalar.copy(out=x_m[:], in_=ps[:])

                # qkv (fp8 DR, bias folded): q per-s, all k packed into kall
                # so the score muls can batch across s
                qk_t, v_t = [], []
                kall = p_qkv.tile([P, 3, D], F16, name="kall", tag="kall", bufs=3)
                for s in range(3):
                    qk = p_qkv.tile([P, D], F16, name=f"q{s}", tag=f"q{s}", bufs=3)
                    vv = p_qkv.tile([P, D], F16, name=f"v{s}", tag=f"v{s}", bufs=3)
                    for g in range(3):
                        ps = ps_mm.tile([P, D], F32, name="mm", tag="mm")
                        col = s * 3 * D + g * D
                        for c2 in range(2):
                            nc.tensor.matmul(ps[:], lhsT=rawT8[s][:, 2 * c2:2 * c2 + 2, :],
                                             rhs=wqkv_sb[:, 2 * c2:2 * c2 + 2, col:col + D],
                                             start=(c2 == 0), stop=(g > 0 and c2 == 1),
                                             perf_mode=DR)
                        if g == 0:
                            # k/v biases handled algebraically (see _fold)
                            nc.tensor.matmul(ps[:], lhsT=onesd[:, :],
                                             rhs=bqkv_sb[:, col:col + D],
                                             start=False, stop=True)
                        if g == 0:
                            nc.scalar.copy(out=qk[:], in_=ps[:])
                        elif g == 1:
                            nc.scalar.copy(out=kall[:, s, :], in_=ps[:])
                        else:
                            # descale v so ctx/ctxT stay at natural magnitude
                            nc.scalar.mul(out=vv[:], in_=ps[:], mul=ISCL)
                    qk_t.append(qk)
                    v_t.append(vv)

                # MLP hidden, TRANSPOSED: hT = relu(sum_s A_s^T @ raw_s^T + bh)
                # (hidden on partitions, so bh rides the Relu eviction as a
                # per-partition ACT bias). Emitted after qkv.
                ps_h = ps_mm.tile([P, D], F32, name="mm", tag="mm")
                for m in range(2):
                    for s in range(3):
                        for c2 in range(2):
                            nc.tensor.matmul(ps_h[:, ts(m)],
                                             lhsT=wh_sb[:, 2 * c2:2 * c2 + 2, s * 256 + m * P: s * 256 + (m + 1) * P],
                                             rhs=rawT8[s][:, 2 * c2:2 * c2 + 2, :],
                                             start=(m == 0 and s == 0 and c2 == 0),
                                             stop=(m == 1 and s == 2 and c2 == 1),
                                             perf_mode=DR)
                hT = p_sm.tile([P, 2, P], F16, name="hT", tag="hT", bufs=3)
                for m in range(2):
                    nc.scalar.activation(out=hT[:, m, :], in_=ps_h[:, ts(m)],
                                         func=AF.Relu, bias=bht_sb[:, m:m + 1])

                # attention scores: fused (q*k -> per-head sum) on DVE, one
                # tensor_tensor_reduce per (q, s, head); 1/sqrt(K) folded into
                # the product scale so the softmax exp runs unscaled.
                sc_t = p_sm.tile([P, 36], F32, name="sc", tag="sc", bufs=3)
                if USE_TTR:
                    junk = p_sm.tile([P, KD], F16, name="scjunk", tag="scjunk", bufs=2)
                    for j in range(9):
                        qi, si = divmod(j, 3)
                        for h in range(H):
                            nc.vector.tensor_tensor_reduce(
                                out=junk[:],
                                in0=qk_t[qi][:, h * KD:(h + 1) * KD],
                                in1=kall[:, si, h * KD:(h + 1) * KD],
                                scale=isk * ISCL * ISCL, scalar=0.0,
                                op0=OP.mult, op1=OP.add,
                                accum_out=sc_t[:, j * 4 + h:j * 4 + h + 1])
                else:
                    # nine 2x-mode muls packed per-qi, then one [P,12,128]
                    # reduce per query row (3 reduces instead of 5)
                    scr3 = p_sm.tile([P, 3, D], F16, name="scr3", tag="scr3", bufs=3)
                    for qi in range(3):
                        for si in range(3):
                            nc.vector.tensor_mul(out=scr3[:, si, :],
                                                 in0=qk_t[qi][:],
                                                 in1=kall[:, si, :])
                        # log-fold k 128->32 with two cheap TT adds, then a
                        # quarter-size 1x-mode reduce (reduce has no 2x uop)
                        v3 = scr3[:].rearrange("p a (h f k) -> p (a h) f k", h=H, f=2)
                        nc.vector.tensor_add(out=v3[:, :, 0, :], in0=v3[:, :, 0, :],
                                             in1=v3[:, :, 1, :])
                        v4 = scr3[:].rearrange("p a (h g f k) -> p (a h) g f k",
                                               h=H, g=2, f=2)
                        nc.vector.tensor_add(out=v4[:, :, 0, 0, :], in0=v4[:, :, 0, 0, :],
                                             in1=v4[:, :, 0, 1, :])
                        nc.vector.tensor_reduce(
                            out=sc_t[:, qi * 12:(qi + 1) * 12],
                            in_=v4[:, :, 0, 0, :],
                            axis=AX.X, op=OP.add)

                # adaptive-weight logits awl = hT^T @ Wa2 + ba2 (PE)
                ps_a = ps_mm.tile([P, D], F32, name="mm", tag="mm")
                for m in range(2):
                    nc.tensor.matmul(ps_a[:, 0:3], lhsT=hT[:, m, :], rhs=wa2_sb[:, m, :],
                                     start=(m == 0), stop=False)
                nc.tensor.matmul(ps_a[:, 0:3], lhsT=onesd[:, :], rhs=ba2_sb[:, :],
                                 start=False, stop=True)
                awl = p_sm.tile([P, 3], F32, name="awl", tag="awl", bufs=3)
                nc.scalar.copy(out=awl[:], in_=ps_a[:, 0:3])

                return dict(r0=r0, x_m=x_m, rawT=rawT, v_t=v_t, sc_t=sc_t, awl=awl)

            def emitA2(st):
                """Softmax + ctx accumulation + adaptive-weight softmax + ctxT."""
                sc_t, awl, v_t = st["sc_t"], st["awl"], st["v_t"]
                # adaptive weights aw = softmax(awl)
                aw_e = p_sm.tile([P, 3], F32, name="awe", tag="awe")
                aw_sum = p_sm.tile([P, 1], F32, name="aws", tag="aws")
                nc.scalar.activation(out=aw_e[:], in_=awl[:], func=AF.Exp,
                                     scale=ISCL, accum_out=aw_sum[:])
                aw_r = p_sm.tile([P, 1], F32, name="awr", tag="awr")
                nc.vector.reciprocal(out=aw_r[:], in_=aw_sum[:])
                aw_t = p_sm.tile([P, 3], F32, name="aw", tag="aw", bufs=6)
                nc.vector.tensor_scalar_mul(out=aw_t[:], in0=aw_e[:], scalar1=aw_r[:])

                # softmax over s (f16 weights; TTR folds 1/sqrt(K) into sc)
                e_t = p_sm.tile([P, 36], F16, name="e", tag="e")
                nc.scalar.activation(out=e_t[:], in_=sc_t[:], func=AF.Exp,
                                     scale=(1.0 if USE_TTR else isk * ISCL * ISCL))
                e4 = e_t[:].rearrange("p (q s h) -> p q s h", q=3, s=3)
                ssum = p_sm.tile([P, 12], F32, name="ssum", tag="ssum")
                ss4 = ssum[:].rearrange("p (q h) -> p q h", q=3)
                nc.vector.tensor_add(out=ss4, in0=e4[:, :, 0, :], in1=e4[:, :, 1, :])
                nc.vector.tensor_add(out=ss4, in0=ss4, in1=e4[:, :, 2, :])
                rinv = p_sm.tile([P, 12], F16, name="rinv", tag="rinv")
                with nc.allow_low_precision(reason="softmax weights fine in fp16"):
                    nc.vector.reciprocal(out=rinv[:], in_=ssum[:])
                attn = p_sm.tile([P, 36], F16, name="attn", tag="attn")
                a4 = attn[:].rearrange("p (q s h) -> p q s h", q=3, s=3)
                rb = rinv[:].rearrange("p (q h) -> p q h", q=3).unsqueeze(2).broadcast_to([P, 3, 3, H])
                nc.vector.tensor_mul(out=a4, in0=e4, in1=rb)

                # ctx for all three queries at once: per s, one broadcast mul
                # over [P, 3q, 512] (v stride-0 across q, attn stride-0 across
                # k), then two batched adds
                ctx3 = p_ctx.tile([P, 3, D], F16, name="ctx3", tag="ctx3", bufs=2)
                ctmp3 = p_sm.tile([P, 3, D], F16, name="ctmp3", tag="ctmp3", bufs=2)
                for si in range(3):
                    dst = ctx3 if si == 0 else ctmp3
                    nc.vector.tensor_mul(
                        out=dst[:].rearrange("p q (k h) -> p q k h", k=KD),
                        in0=v_t[si][:].rearrange("p (k h) -> p k h", k=KD)
                            .unsqueeze(1).broadcast_to([P, 3, KD, H]),
                        in1=a4[:, :, si, :].unsqueeze(2).broadcast_to([P, 3, KD, H]))
                    if si > 0:
                        nc.vector.tensor_add(out=ctx3[:], in0=ctx3[:], in1=ctmp3[:])

                # ctx^T via DMA XBAR transpose (SBUF->SBUF, 2-byte dtype)
                ctxT = []
                for qi in range(3):
                    ct = p_ctx.tile([P, 4, P], F16, name=f"ctxT{qi}", tag=f"ctxT{qi}", bufs=5)
                    nc.sync.dma_start(out=ct[:, :, :], in_=ctx3[:, qi, :],
                                      transpose=True)
                    ctxT.append(ct)

                st["aw_t"] = aw_t
                st["ctxT"] = ctxT
                return st

            def emitB1(st):
                """o-proj (+frontier/cross proj residual in PSUM), LN1 batched
                stage-wise across the 3 modalities, weighted sum."""
                x_m, aw_t, rawT = st["x_m"], st["aw_t"], st["rawT"]
                ctxT = st["ctxT"]
                yins = []
                for qi in range(3):
                    ps = ps_mm.tile([P, D], F32, name="mm", tag="mm")
                    for c in range(4):
                        nc.tensor.matmul(ps[:], lhsT=ctxT[qi][:, c, :], rhs=wo_sb[:, c, :],
                                         start=(c == 0), stop=False)
                    # accumulate x_qi = raw_qi @ W_qi directly into the same bank
                    # (qi=2 recomputes mp on the idle PE instead of a DVE add)
                    for c in range(4):
                        nc.tensor.matmul(ps[:], lhsT=rawT[qi][:, c, :],
                                         rhs=wproj_sb[:, c, ts(qi, D)],
                                         start=False, stop=False)
                    nc.tensor.matmul(ps[:], lhsT=onesd[:, :], rhs=bxo_sb[:, ts(qi, D)],
                                     start=False, stop=True)
                    yins.append(ps[:])
                # LN1 x3 with a shared [P,6] (mean,var)x3 tile so the serial
                # rstd chain is one strided Ln + one Exp + one nmr stt
                mvs = p_sm.tile([P, 6], F32, name="mvs", tag="mvs", bufs=3)
                for qi in range(3):
                    st6 = p_sm.tile([P, 6], F32, name="st6", tag="st6", bufs=6)
                    nc.vector.bn_stats(out=st6[:], in_=yins[qi])
                    nc.vector.bn_aggr(out=mvs[:, 2 * qi:2 * qi + 2], in_=st6[:])
                mvv = mvs[:].rearrange("p (q t) -> p q t", q=3)
                lnt3 = p_sm.tile([P, 3], F32, name="lnt3", tag="lnt3", bufs=3)
                nc.scalar.activation(out=lnt3[:], in_=mvv[:, :, 1], func=AF.Ln,
                                     bias=eps_sb[:], scale=1.0)
                rstd3 = p_sm.tile([P, 3], F32, name="rstd3", tag="rstd3", bufs=3)
                nc.scalar.activation(out=rstd3[:], in_=lnt3[:], func=AF.Exp, scale=-0.5)
                nmr3 = p_sm.tile([P, 3], F32, name="nmr3", tag="nmr3", bufs=3)
                nc.vector.scalar_tensor_tensor(out=nmr3[:], in0=mvv[:, :, 0], scalar=-1.0,
                                               in1=rstd3[:], op0=OP.mult, op1=OP.mult)
                y_t = []
                for qi in range(3):
                    yq = p_y.tile([P, D], F16, name=f"y{qi}", tag=f"y{qi}")
                    ln_apply(yins[qi], yq, rstd3[:, qi:qi + 1], nmr3[:, qi:qi + 1],
                             0 if need_aff1 else None)
                    y_t.append(yq)

                # weighted = sum_q aw_q * y_q (GpSimd)
                w_eng = nc.gpsimd if WEIGHTED_ON_GPSIMD else nc.vector
                w_t = p_w.tile([P, D], F16, name="w", tag="w", bufs=3)
                nc.scalar.mul(out=w_t[:], in_=y_t[0][:], mul=aw_t[:, 0:1])
                for qi in (1, 2):
                    w_eng.scalar_tensor_tensor(out=w_t[:], in0=y_t[qi][:],
                                               scalar=aw_t[:, qi:qi + 1], in1=w_t[:],
                                               op0=OP.mult, op1=OP.add)
                st["w_t"] = w_t
                return st

            def emitB2(st):
                """Gate path: weighted^T, gate matmul + sigmoid chain, z, LN2,
                store. Runs one pipeline stage after B1 so the PE's wT/gate
                matmuls never wait on B1's LN chain."""
                r0, x_m, w_t = st["r0"], st["x_m"], st["w_t"]
                # weighted^T: packed PE transpose (w already fp16)
                tpw = ps_t16.tile([P, D], F16, name="tp16", tag="tp16")
                for c in range(4):
                    nc.tensor.matmul(tpw[:, ts(c)], lhsT=w_t[:, ts(c)], rhs=ident16[:],
                                     is_transpose=True, start=(c == 0), stop=(c == 3))
                wT = p_w.tile([P, D], F16, name="wT", tag="wT")
                nc.scalar.copy(out=wT[:], in_=tpw[:])

                # gate = sigmoid(w @ Wg + bg) = exp(-ln(1+exp(-g)))
                ps_g = ps_mm.tile([P, D], F32, name="mm", tag="mm")
                for c in range(4):
                    nc.tensor.matmul(ps_g[:], lhsT=wT[:, ts(c)], rhs=wg_sb[:, c, :],
                                     start=(c == 0), stop=False)
                nc.tensor.matmul(ps_g[:], lhsT=onesd[:, :], rhs=bg_sb[:, :],
                                 start=False, stop=True)
                eg = p_w.tile([P, D], F32, name="eg", tag="eg")
                nc.scalar.activation(out=eg[:], in_=ps_g[:], func=AF.Exp, scale=-1.0)
                lg = p_w.tile([P, D], F32, name="lg", tag="lg")
                nc.scalar.activation(out=lg[:], in_=eg[:], func=AF.Ln, bias=1.0)
                gate = p_w.tile([P, D], F16, name="gate", tag="gate")
                nc.scalar.activation(out=gate[:], in_=lg[:], func=AF.Exp, scale=-1.0)

                # z = mp + gate*w ; out = LN2(z)
                z_eng = nc.gpsimd if Z_ON_GPSIMD else nc.vector
                z_t = p_w.tile([P, D], F16, name="z", tag="z")
                z_eng.tensor_mul(out=z_t[:], in0=gate[:], in1=w_t[:])
                nc.vector.tensor_add(out=z_t[:], in0=z_t[:], in1=x_m[:])
                rstd2, nmr2 = ln_stats(z_t[:])
                out_t = p_w.tile([P, D], F32, name="outt", tag="outt")
                ln_apply(z_t[:], out_t, rstd2, nmr2, 2 if need_aff2 else None)
                nc.sync.dma_start(out=out_ext[r0:r0 + P, :], in_=out_t[:])

            # software pipeline: DMA(i+2), A1(i), A2(i-1), B1(i-3), B2(i-4) —
            # B1 trails A2 by two iterations so the o-proj LDWEIGHTS never
            # waits on the scores->softmax->ctx->ctxT cross-engine chain.
            from collections import deque
            dma_q = {}
            q1, q2, q3 = deque(), deque(), deque()
            for i in range(ntiles):
                if i == 0:
                    dma_q[0] = emitDMA(0)
                    if ntiles > 1:
                        dma_q[1] = emitDMA(1)
                q1.append(emitA1(i, dma_q.pop(i)))
                if len(q1) > 1:
                    q2.append(emitA2(q1.popleft()))
                if i + 2 < ntiles:
                    dma_q[i + 2] = emitDMA(i + 2)
                if len(q2) > 2:
                    q3.append(emitB1(q2.popleft()))
                if len(q3) > 1:
                    emitB2(q3.popleft())
            q2.append(emitA2(q1.popleft()))
            while q2:
                q3.append(emitB1(q2.popleft()))
                if len(q3) > 1:
                    emitB2(q3.popleft())
            while q3:
                emitB2(q3.popleft())
    nc.finalize()
    return nc


def kernel(**inputs):
    global LAST_EXEC_TIME_NS, LAST_RESULTS
    inputs = {k: np.ascontiguousarray(np.asarray(v)) for k, v in inputs.items()}
    Bfull = inputs['frontier'].shape[0]
    assert Bfull % (NCORES * P) == 0
    R = Bfull // NCORES

    folded, aff = _fold(inputs)
    need_aff1 = not (np.allclose(aff[0], 1.0) and np.allclose(aff[1], 0.0))
    need_aff2 = not (np.allclose(aff[2], 1.0) and np.allclose(aff[3], 0.0))
    nc = _build(R, need_aff1, need_aff2)

    x16 = {n: inputs[n].astype(np.float16) for n in ("frontier", "cross_robot", "map_feat")}
    in_maps = []
    for c in range(NCORES):
        m = {n: x16[n][c * R:(c + 1) * R] for n in ("frontier", "cross_robot", "map_feat")}
        m.update(folded)
        if need_aff1 or need_aff2:
            m["aff"] = aff
        in_maps.append(m)

    trace = bool(os.environ.get("KERNEL_TRACE"))
    res = run_bass_kernel_spmd(nc, in_maps, core_ids=list(range(NCORES)), trace=trace)
    LAST_EXEC_TIME_NS = res.exec_time_ns
    LAST_RESULTS = res
    out = np.concatenate([res.results[c]["out"] for c in range(NCORES)], axis=0)
    return out.astype(np.float32)

